# revision 1
# baseline (speedup 1.0000x reference)
"""Causal self-attention (B=4, T=2048, D=1024, H=16) on 8 trn2 NeuronCores.

Sharding: data-parallel over batch (4) x tensor-parallel over heads (2 groups
of 8 heads). Core c handles batch c//2 and head-group c%2. Each core:
  1. qkv projection for its 512 qkv columns (8 heads x 64 x {q,k,v})
  2. causal attention for its 8 heads (flash-style, transposed S^T tiles,
     unnormalized exp + ones-column row sums, normalized via gpsimd
     partition-broadcast of the reciprocal row sums)
  3. partial out-projection  y_local @ w_out[rows of its heads]
Host sums the two head-group partials per batch (the "all-reduce").

Schedule: i-chunk-major. The attention block for chunk ic overlaps (on the
otherwise-idle engines) the k/v projection of chunk ic+1 and the
out-projection of chunk ic-1, keeping TensorE dense while ScalarE (exp)
is the steady-state bottleneck. Diagonal S^T tiles are width-trimmed to
skip the fully-masked half.

Matmul inputs are float32r-typed (full-rate fp32 mode on the PE array);
walrus requires every producer of an f32r matmul operand to emit f32r.
"""

import numpy as np

import concourse.bass as bass
import concourse.mybir as mybir
from concourse import bacc, library_config
from concourse.tile import TileContext
from concourse.bass_utils import run_bass_kernel_spmd

F32 = mybir.dt.float32
P = 128

B, T, D, H, HD = 4, 2048, 1024, 16, 64
HLOC = H // 2          # heads per core
W = HLOC * HD          # 512: local qkv width per section
N_CORES = 8


def build_nc(T=T, D=D, mm_dtype=mybir.dt.float32r, reps=1,
             pt_bufs=5, pss_bufs=2, psg_bufs=2, h0_0=4, h0_n=2, pv_lag=4, psy_bufs=2, rr_bufs=1,
             share_psy=False, skip_attention=False):
    S_D = D // P           # contraction slices (8)
    NPAIR = HLOC // 2      # head pairs (4)
    TT = T // P            # 128-row tiles (16)
    IC = 512               # i-chunk (moving free dim)
    NIC = T // IC          # 4
    JPC = IC // P          # j-tiles per i-chunk (4)
    NEC = D // 512         # out-proj column chunks (2)
    scale = float(1.0 / np.sqrt(HD))
    MMD = mm_dtype

    nc = bacc.Bacc("TRN2", target_bir_lowering=False, debug=False,
                   num_devices=N_CORES)

    def mm(out, lhsT, rhs, start, stop):
        nc.tensor.matmul(out, lhsT=lhsT, rhs=rhs, start=start, stop=stop)

    xT = nc.declare_dram_parameter("xT", [D, T], F32, isOutput=False)
    wq = nc.declare_dram_parameter("wq", [D, W], F32, isOutput=False)
    wk = nc.declare_dram_parameter("wk", [D, W], F32, isOutput=False)
    wv = nc.declare_dram_parameter("wv", [D, W], F32, isOutput=False)
    wo = nc.declare_dram_parameter("wo", [W, D], F32, isOutput=False)
    out = nc.declare_dram_parameter("out", [T, D], F32, isOutput=True)

    xT_r = xT.rearrange("(s p) t -> p s t", p=P).bitcast(MMD)   # [128, S_D, T]
    wq_r = wq.rearrange("(s p) n -> p s n", p=P).bitcast(MMD)   # [128, S_D, W]
    wk_r = wk.rearrange("(s p) n -> p s n", p=P).bitcast(MMD)
    wv_r = wv.rearrange("(s p) n -> p s n", p=P).bitcast(MMD)
    wo_r = wo.rearrange("(m p) e -> p m e", p=P).bitcast(MMD)   # [128, NPAIR, D]
    out_r = out.rearrange("(t p) e -> p t e", p=P)              # [128, TT, D]

    with TileContext(nc) as tc:
        nc.gpsimd.load_library(library_config.attn)
        with (
            tc.tile_pool(name="const", bufs=1) as const_pool,
            tc.tile_pool(name="persist", bufs=1) as persist,
            tc.tile_pool(name="w1", bufs=1) as w1_pool,
            tc.tile_pool(name="xp", bufs=1) as x_pool,
            tc.tile_pool(name="qTp", bufs=1) as qT_pool,
            tc.tile_pool(name="yTp", bufs=2) as yT_pool,
            tc.tile_pool(name="ptp", bufs=pt_bufs) as pt_pool,
            tc.tile_pool(name="recp", bufs=rr_bufs) as rec_pool,
            tc.tile_pool(name="recbp", bufs=rr_bufs) as recb_pool,
            tc.tile_pool(name="outp", bufs=2) as out_pool,
            tc.tile_pool(name="ps_g", bufs=psg_bufs, space="PSUM") as psum_g,
            tc.tile_pool(name="ps_s", bufs=pss_bufs, space="PSUM") as psum_s,
            tc.tile_pool(name="ps_y", bufs=(1 if share_psy else psy_bufs),
                         space="PSUM") as psum_y,
        ):
            # staircase causal masks for the packed diagonal units:
            # mask_a = [stair(512) | stair(384)], mask_b = [stair(256) |
            # stair(128)]; stair(w)[p, f] = 1.0 if p <= f else 0.0
            mask_a = const_pool.tile([P, 2 * IC], F32)
            mask_b = const_pool.tile([P, IC], F32)
            ones_f32 = const_pool.tile([P, max(TT * HLOC, HD)], F32)
            nc.gpsimd.memset(ones_f32[:], 1.0)
            nc.gpsimd.memset(mask_a[:], 1.0)
            nc.gpsimd.memset(mask_b[:], 1.0)
            for mt, seg in ((mask_a, (512, 384)), (mask_b, (256, 128))):
                off = 0
                for w in seg:
                    nc.gpsimd.affine_select(
                        out=mt[:, off:off + w], in_=mt[:, off:off + w],
                        compare_op=mybir.AluOpType.is_ge, fill=0.0,
                        base=0, pattern=[[1, w]], channel_multiplier=-1,
                    )
                    off += w

            # persistent activations: k^T as [pair-row, pair, T]; v natural
            # per (tile, head) with an appended ones column (row-sum trick)
            kT = persist.tile([P, NPAIR, T], MMD)
            v_sb = persist.tile([P, TT, HLOC, HD + 1], MMD)
            nc.vector.tensor_copy(
                v_sb[:, :, :, HD:HD + 1],
                ones_f32[:, 0:TT * HLOC].rearrange(
                    "p (a b) -> p a b", b=HLOC)[:, :, :, None])

            wq_sb = w1_pool.tile([P, S_D, W], MMD)
            wk_sb = w1_pool.tile([P, S_D, W], MMD)
            wv_sb = w1_pool.tile([P, S_D, W], MMD)
            wo_sb = w1_pool.tile([P, NPAIR, D], MMD)

            for _rep in range(reps):
                nc.sync.dma_start(wq_sb[:], wq_r)
                xc0 = x_pool.tile([P, S_D, IC], MMD, tag="xc", name="xc")
                nc.sync.dma_start(xc0[:], xT_r[:, :, 0:IC])
                nc.sync.dma_start(wk_sb[:], wk_r)
                nc.sync.dma_start(wv_sb[:], wv_r)
                nc.sync.dma_start(wo_sb[:], wo_r)

                qTc = {0: None}   # ic -> q^T tile [P, NPAIR, IC]
                xcs = {0: xc0}

                def group_q(ic, m):
                    ps = psum_g.tile([P, IC], F32, tag="psg", name="psg")
                    for s in range(S_D):
                        mm(ps, wq_sb[:, s, m * P:(m + 1) * P], xcs[ic][:, s, :],
                           s == 0, s == S_D - 1)
                    nc.vector.tensor_copy(qTc[ic][:, m, :], ps)

                def group_k(ic, m):
                    ps = psum_g.tile([P, IC], F32, tag="psg", name="psg")
                    for s in range(S_D):
                        mm(ps, wk_sb[:, s, m * P:(m + 1) * P], xcs[ic][:, s, :],
                           s == 0, s == S_D - 1)
                    nc.vector.tensor_copy(kT[:, m, ic * IC:(ic + 1) * IC], ps)

                def group_v(ic, itl):
                    tt = ic * JPC + itl
                    ps = psum_g.tile([P, IC], F32, tag="psg", name="psg")
                    for s in range(S_D):
                        mm(ps, xcs[ic][:, s, itl * P:(itl + 1) * P],
                           wv_sb[:, s, :], s == 0, s == S_D - 1)
                    nc.vector.tensor_copy(
                        v_sb[:, tt, :, 0:HD],
                        ps.rearrange("p (h d) -> p h d", d=HD))

                def normalize(pend):
                    ps_y, h, m, po, ic, ytile = pend
                    rec = rec_pool.tile([1, IC], F32, tag="rec", name="rec")
                    nc.vector.reciprocal(rec[:], ps_y[HD:HD + 1, :])
                    recb = recb_pool.tile([HD, IC], F32, tag="recb",
                                          name="recb")
                    nc.gpsimd.partition_broadcast(recb[:], rec[:])
                    nc.vector.tensor_mul(ytile[po:po + HD, m, :],
                                         ps_y[0:HD, :], recb[:])

                def out_proj(ic, ytile):
                    for itl in range(JPC):
                        for ec in range(NEC):
                            ps_o = psum_g.tile([P, 512], F32, tag="psg",
                                               name="psg")
                            for m in range(NPAIR):
                                mm(ps_o, ytile[:, m, itl * P:(itl + 1) * P],
                                   wo_sb[:, m, ec * 512:(ec + 1) * 512],
                                   m == 0, m == NPAIR - 1)
                            o_t = out_pool.tile([P, 512], F32, tag="ot",
                                                name="ot")
                            nc.vector.tensor_copy(o_t[:], ps_o)
                            nc.sync.dma_start(
                                out_r[:, ic * JPC + itl,
                                      ec * 512:(ec + 1) * 512], o_t[:])

                # phase 1 for chunk 0 (nothing to overlap with yet)
                qTc[0] = qT_pool.tile([P, NPAIR, IC], MMD, tag="qT", name="qT")
                for m in range(NPAIR):
                    group_q(0, m)
                for m in range(NPAIR):
                    group_k(0, m)
                for itl in range(JPC):
                    group_v(0, itl)

                pending = None
                for ic in range(NIC):
                    # prefetch next x chunk; its k/v projection groups
                    # interleave into this attention block below
                    kv_todo = []
                    if ic + 1 < NIC:
                        nxc = x_pool.tile([P, S_D, IC], MMD, tag="xc",
                                          name="xc")
                        xcs[ic + 1] = nxc
                        nc.sync.dma_start(
                            nxc[:], xT_r[:, :, (ic + 1) * IC:(ic + 2) * IC])
                        kv_todo = ([lambda m=m: group_k(ic + 1, m)
                                    for m in range(NPAIR)]
                                   + [lambda i=i: group_v(ic + 1, i)
                                      for i in range(JPC)])
                    h0 = h0_0 if ic == 0 else h0_n
                    ytile = yT_pool.tile([P, NPAIR, IC], MMD, tag="yT",
                                         name="yT")

                    if skip_attention:  # timing probe: phases 1+3 only
                        nc.vector.tensor_copy(
                            ytile[:], kT[:, :, ic * IC:(ic + 1) * IC])
                        for fn in kv_todo:
                            fn()
                        if ic + 1 < NIC:
                            qTc[ic + 1] = qT_pool.tile([P, NPAIR, IC], MMD,
                                                       tag="qT", name="qT")
                            for m in range(NPAIR):
                                group_q(ic + 1, m)
                        out_proj(ic, ytile)
                        continue
                    for h in range(HLOC):
                        m, po = h // 2, (h % 2) * HD
                        njb = JPC * (ic + 1)
                        ps_y = (psum_g if share_psy else psum_y).tile(
                            [HD + 1, IC], F32,
                            tag="psg" if share_psy else "psy", name="psy")
                        # units pack 2 j-tiles per PSUM double-bank so one
                        # ACTIVATE (and one mask multiply) covers both.
                        # Each entry: (mask, [(psum_off, i_off, width, jb)..])
                        units = []
                        for p2 in range(ic * JPC // 2):
                            units.append((None, [(0, 0, IC, 2 * p2),
                                                 (IC, 0, IC, 2 * p2 + 1)]))
                        d = ic * JPC
                        units.append((mask_a, [(0, 0, 512, d),
                                               (512, P, 384, d + 1)]))
                        units.append((mask_b, [(0, 2 * P, 256, d + 2),
                                               (256, 3 * P, 128, d + 3)]))
                        work = []  # (pt2, parts)
                        for ui, (msk, parts) in enumerate(units):
                            ps2 = psum_s.tile([P, 2 * IC], F32, tag="pss",
                                              name="pss")
                            for (off, i0, wdt, jb) in parts:
                                r_off = max(0, (jb - JPC * ic) * P)
                                mm(ps2[:, off:off + wdt],
                                   kT[po:po + HD, m, jb * P:(jb + 1) * P],
                                   qTc[ic][po:po + HD, m, r_off:IC],
                                   True, True)
                            ext = parts[-1][0] + parts[-1][2]
                            pt2 = pt_pool.tile([P, 2 * IC], MMD, tag="pt",
                                               name="pt")
                            nc.scalar.activation(
                                pt2[:, 0:ext], ps2[:, 0:ext],
                                mybir.ActivationFunctionType.Exp, scale=scale)
                            if msk is not None:
                                nc.vector.tensor_mul(pt2[:, 0:ext],
                                                     pt2[:, 0:ext],
                                                     msk[:, 0:ext])
                            work.append((pt2, parts))
                            if ui == 0 and pending is not None:
                                # out-proj of the previous chunk, deferred so
                                # its matmuls don't wait on the DVE normalize
                                out_proj(pending[0], pending[1])
                                pending = None
                            if len(work) > pv_lag:
                                ppt, pparts = work[-1 - pv_lag]
                                for (off, i0, wdt, jb) in pparts:
                                    mm(ps_y[:, IC - wdt:IC],
                                       v_sb[:, jb, h, :], ppt[:, off:off + wdt],
                                       jb == 0, False)
                        for (ppt, pparts) in work[-min(pv_lag, len(work)):]:
                            for (off, i0, wdt, jb) in pparts:
                                mm(ps_y[:, IC - wdt:IC], v_sb[:, jb, h, :],
                                   ppt[:, off:off + wdt], jb == 0,
                                   jb == njb - 1)
                        normalize((ps_y, h, m, po, ic, ytile))
                        if h == HLOC - 1:
                            pending = (ic, ytile)
                        # interleave next chunk's k/v projection groups
                        if h >= h0 and kv_todo:
                            n_left = HLOC - h
                            n_emit = max(1, -(-len(kv_todo) // n_left))
                            for _ in range(min(n_emit, len(kv_todo))):
                                kv_todo.pop(0)()
                    for fn in kv_todo:
                        fn()
                    if ic + 1 < NIC:  # q projection for next chunk
                        qTc[ic + 1] = qT_pool.tile([P, NPAIR, IC], MMD,
                                                   tag="qT", name="qT")
                        for m in range(NPAIR):
                            group_q(ic + 1, m)
                if not skip_attention:
                    out_proj(pending[0], pending[1])
                    pending = None

    nc.compile()
    return nc


def shard_inputs(x, w_qkv, w_out):
    """Full inputs -> list of 8 per-core input maps."""
    in_maps = []
    for c in range(N_CORES):
        b, g = c // 2, c % 2
        hsl = slice(g * W, (g + 1) * W)
        in_maps.append({
            "xT": np.ascontiguousarray(x[b].T),
            "wq": np.ascontiguousarray(w_qkv[:, 0 * D:1 * D][:, hsl]),
            "wk": np.ascontiguousarray(w_qkv[:, 1 * D:2 * D][:, hsl]),
            "wv": np.ascontiguousarray(w_qkv[:, 2 * D:3 * D][:, hsl]),
            "wo": np.ascontiguousarray(w_out[hsl, :]),
        })
    return in_maps


_NC_CACHE = {}


def _run_spmd(nc, in_maps):
    """One execution on cores 0-7. First call goes through
    run_bass_kernel_spmd (which jits + compiles the NEFF via the axon/PJRT
    path); the compiled executable is cached so repeat kernel() calls skip
    the multi-minute recompile."""
    import os
    os.environ.setdefault("BASS_NEVER_TRACE", "1")  # no NTFF hook here
    fn_pack = _NC_CACHE.get("fn")
    if fn_pack is None:
        import jax
        from jax.sharding import Mesh, PartitionSpec, NamedSharding
        try:
            from jax.experimental.shard_map import shard_map
        except ImportError:
            from jax import shard_map
        from concourse import bass2jax

        bass2jax.install_neuronx_cc_hook()
        pname = nc.partition_id_tensor.name if nc.partition_id_tensor else None
        in_names, out_names, out_avals = [], [], []
        for alloc in nc.m.functions[0].allocations:
            if not isinstance(alloc, mybir.MemoryLocationSet):
                continue
            name = alloc.memorylocations[0].name
            if alloc.kind == "ExternalInput":
                if name != pname:
                    in_names.append(name)
            elif alloc.kind == "ExternalOutput":
                out_names.append(name)
                out_avals.append(jax.core.ShapedArray(
                    tuple(alloc.tensor_shape), mybir.dt.np(alloc.dtype)))
        all_in = list(in_names) + list(out_names) + ([pname] if pname else [])

        def _body(*args):
            operands = list(args)
            if pname is not None:
                operands.append(bass2jax.partition_id_tensor())
            return tuple(bass2jax._bass_exec_p.bind(
                *operands, out_avals=tuple(out_avals),
                in_names=tuple(all_in), out_names=tuple(out_names),
                lowering_input_output_aliases=(),
                sim_require_finite=True, sim_require_nnan=True, nc=nc))

        mesh = Mesh(np.asarray(jax.devices()[:N_CORES]), ("core",))
        n_io = len(in_names) + len(out_names)
        fn = jax.jit(shard_map(_body, mesh=mesh,
                               in_specs=(PartitionSpec("core"),) * n_io,
                               out_specs=(PartitionSpec("core"),) * len(out_names),
                               check_rep=False), keep_unused=True)
        sharding = NamedSharding(mesh, PartitionSpec("core"))
        fn_pack = (fn, in_names, out_names, out_avals, sharding, jax)
        _NC_CACHE["fn"] = fn_pack
    fn, in_names, out_names, out_avals, sharding, jax = fn_pack
    concat_in = [np.concatenate([np.asarray(m[n]) for m in in_maps], axis=0)
                 for n in in_names]
    concat_zeros = [np.zeros((N_CORES * a.shape[0], *a.shape[1:]), a.dtype)
                    for a in out_avals]
    dev_args = [jax.device_put(a, sharding) for a in concat_in + concat_zeros]
    outs = fn(*dev_args)
    jax.block_until_ready(outs)
    return [
        {n: np.asarray(outs[i]).reshape(N_CORES, *out_avals[i].shape)[c]
         for i, n in enumerate(out_names)}
        for c in range(N_CORES)
    ]


def kernel(x, w_qkv, w_out, **run_kwargs):
    x = np.asarray(x, dtype=np.float32)
    w_qkv = np.asarray(w_qkv, dtype=np.float32)
    w_out = np.asarray(w_out, dtype=np.float32)
    if "nc" not in _NC_CACHE:
        _NC_CACHE["nc"] = build_nc()
    nc = _NC_CACHE["nc"]
    in_maps = shard_inputs(x, w_qkv, w_out)
    try:
        results = _run_spmd(nc, in_maps)
    except Exception:
        res = run_bass_kernel_spmd(nc, in_maps, core_ids=list(range(N_CORES)),
                                   **run_kwargs)
        _NC_CACHE["last_results"] = res
        results = res.results
    outs = [r["out"] for r in results]
    full = np.stack([outs[2 * b] + outs[2 * b + 1] for b in range(B)], axis=0)
    return full


if __name__ == "__main__":
    rng = np.random.default_rng(0)
    x = rng.standard_normal((B, T, D), dtype=np.float32)
    w_qkv = (rng.standard_normal((D, 3 * D), dtype=np.float32) / np.sqrt(D))
    w_out = (rng.standard_normal((D, D), dtype=np.float32) / np.sqrt(D))
    y = kernel(x, w_qkv, w_out)
    print("out", y.shape, y.dtype, float(np.abs(y).mean()))



# revision 2
# speedup vs baseline: 1.0556x; 1.0556x over previous
"""Causal self-attention (B=4, T=2048, D=1024, H=16) on 8 trn2 NeuronCores.

Sharding: data-parallel over batch (4) x tensor-parallel over heads (2 groups
of 8 heads). Core c handles batch c//2 and head-group c%2. Each core:
  1. qkv projection for its 512 qkv columns (8 heads x 64 x {q,k,v})
  2. causal attention for its 8 heads (flash-style, transposed S^T tiles,
     unnormalized exp + ones-column row sums, normalized via gpsimd
     partition-broadcast of the reciprocal row sums)
  3. partial out-projection  y_local @ w_out[rows of its heads]
Host sums the two head-group partials per batch (the "all-reduce").

Schedule: i-chunk-major. The attention block for chunk ic overlaps (on the
otherwise-idle engines) the k/v projection of chunk ic+1 and the
out-projection of chunk ic-1, keeping TensorE dense while ScalarE (exp)
runs concurrently. Diagonal S^T tiles are width-trimmed to skip the
fully-masked half.

All matmul operands are bf16 (inputs converted on host): same 1 cycle/row
PE rate as float32r for wide matmuls but no 4x penalty on the narrow
(<256-col) diagonal tiles, half the DMA traffic, and 2x DVE rate on the
bf16 elementwise ops. Causal masking multiplies only the 128-wide partial
prefix of each diagonal tile (the rest is fully unmasked), with a single
shared [128,128] staircase mask.
"""

import numpy as np
import ml_dtypes

import concourse.bass as bass
import concourse.mybir as mybir
from concourse import bacc, library_config
from concourse.tile import TileContext
from concourse.bass_utils import run_bass_kernel_spmd

F32 = mybir.dt.float32
BF16 = mybir.dt.bfloat16
NP_BF16 = ml_dtypes.bfloat16
P = 128

B, T, D, H, HD = 4, 2048, 1024, 16, 64
HLOC = H // 2          # heads per core
W = HLOC * HD          # 512: local qkv width per section
N_CORES = 8


def build_nc(T=T, D=D, mm_dtype=BF16, reps=1,
             pt_bufs=5, pss_bufs=2, psg_bufs=2, h0_0=4, h0_n=2, pv_lag=4, psy_bufs=2, rr_bufs=1,
             share_psy=False, skip_attention=False):
    S_D = D // P           # contraction slices (8)
    NPAIR = HLOC // 2      # head pairs (4)
    TT = T // P            # 128-row tiles (16)
    IC = 512               # i-chunk (moving free dim)
    NIC = T // IC          # 4
    JPC = IC // P          # j-tiles per i-chunk (4)
    NEC = D // 512         # out-proj column chunks (2)
    scale = float(1.0 / np.sqrt(HD))
    MMD = mm_dtype

    nc = bacc.Bacc("TRN2", target_bir_lowering=False, debug=False,
                   num_devices=N_CORES)

    def mm(out, lhsT, rhs, start, stop):
        nc.tensor.matmul(out, lhsT=lhsT, rhs=rhs, start=start, stop=stop)

    xT = nc.declare_dram_parameter("xT", [D, T], MMD, isOutput=False)
    wq = nc.declare_dram_parameter("wq", [D, W], MMD, isOutput=False)
    wk = nc.declare_dram_parameter("wk", [D, W], MMD, isOutput=False)
    wv = nc.declare_dram_parameter("wv", [D, W], MMD, isOutput=False)
    wo = nc.declare_dram_parameter("wo", [W, D], MMD, isOutput=False)
    out = nc.declare_dram_parameter("out", [T, D], F32, isOutput=True)

    xT_r = xT.rearrange("(s p) t -> p s t", p=P)   # [128, S_D, T]
    wq_r = wq.rearrange("(s p) n -> p s n", p=P)   # [128, S_D, W]
    wk_r = wk.rearrange("(s p) n -> p s n", p=P)
    wv_r = wv.rearrange("(s p) n -> p s n", p=P)
    wo_r = wo.rearrange("(m p) e -> p m e", p=P)   # [128, NPAIR, D]
    out_r = out.rearrange("(t p) e -> p t e", p=P)              # [128, TT, D]

    with TileContext(nc) as tc:
        nc.gpsimd.load_library(library_config.attn)
        with (
            tc.tile_pool(name="const", bufs=1) as const_pool,
            tc.tile_pool(name="persist", bufs=1) as persist,
            tc.tile_pool(name="w1", bufs=1) as w1_pool,
            tc.tile_pool(name="xp", bufs=1) as x_pool,
            tc.tile_pool(name="qTp", bufs=1) as qT_pool,
            tc.tile_pool(name="yTp", bufs=2) as yT_pool,
            tc.tile_pool(name="ptp", bufs=pt_bufs) as pt_pool,
            tc.tile_pool(name="recp", bufs=rr_bufs) as rec_pool,
            tc.tile_pool(name="recbp", bufs=rr_bufs) as recb_pool,
            tc.tile_pool(name="outp", bufs=2) as out_pool,
            tc.tile_pool(name="ps_g", bufs=psg_bufs, space="PSUM") as psum_g,
            tc.tile_pool(name="ps_s", bufs=pss_bufs, space="PSUM") as psum_s,
            tc.tile_pool(name="ps_y", bufs=(1 if share_psy else psy_bufs),
                         space="PSUM") as psum_y,
        ):
            # single staircase mask: stair[p, f] = 1.0 if p <= f else 0.0.
            # Every diagonal S^T part only needs masking on its first 128
            # columns (past that, all 128 j-rows are causally valid).
            stair = const_pool.tile([P, P], MMD)
            ones_bf = const_pool.tile([P, max(TT * HLOC, HD)], MMD)
            nc.gpsimd.memset(ones_bf[:], 1.0)
            nc.gpsimd.memset(stair[:], 1.0)
            nc.gpsimd.affine_select(
                out=stair[:], in_=stair[:],
                compare_op=mybir.AluOpType.is_ge, fill=0.0,
                base=0, pattern=[[1, P]], channel_multiplier=-1,
            )

            # persistent activations: k^T as [pair-row, pair, T]; v natural
            # per (tile, head) with an appended ones column (row-sum trick)
            kT = persist.tile([P, NPAIR, T], MMD)
            v_sb = persist.tile([P, TT, HLOC, HD + 1], MMD)
            nc.vector.tensor_copy(
                v_sb[:, :, :, HD:HD + 1],
                ones_bf[:, 0:TT * HLOC].rearrange(
                    "p (a b) -> p a b", b=HLOC)[:, :, :, None])

            wq_sb = w1_pool.tile([P, S_D, W], MMD)
            wk_sb = w1_pool.tile([P, S_D, W], MMD)
            wv_sb = w1_pool.tile([P, S_D, W], MMD)
            wo_sb = w1_pool.tile([P, NPAIR, D], MMD)

            for _rep in range(reps):
                nc.sync.dma_start(wq_sb[:], wq_r)
                xc0 = x_pool.tile([P, S_D, IC], MMD, tag="xc", name="xc")
                nc.sync.dma_start(xc0[:], xT_r[:, :, 0:IC])
                nc.sync.dma_start(wk_sb[:], wk_r)
                nc.sync.dma_start(wv_sb[:], wv_r)
                nc.sync.dma_start(wo_sb[:], wo_r)

                qTc = {0: None}   # ic -> q^T tile [P, NPAIR, IC]
                xcs = {0: xc0}

                def group_q(ic, m):
                    ps = psum_g.tile([P, IC], F32, tag="psg", name="psg")
                    for s in range(S_D):
                        mm(ps, wq_sb[:, s, m * P:(m + 1) * P], xcs[ic][:, s, :],
                           s == 0, s == S_D - 1)
                    nc.vector.tensor_copy(qTc[ic][:, m, :], ps)

                def group_k(ic, m):
                    ps = psum_g.tile([P, IC], F32, tag="psg", name="psg")
                    for s in range(S_D):
                        mm(ps, wk_sb[:, s, m * P:(m + 1) * P], xcs[ic][:, s, :],
                           s == 0, s == S_D - 1)
                    nc.vector.tensor_copy(kT[:, m, ic * IC:(ic + 1) * IC], ps)

                def group_v(ic, itl):
                    tt = ic * JPC + itl
                    ps = psum_g.tile([P, IC], F32, tag="psg", name="psg")
                    for s in range(S_D):
                        mm(ps, xcs[ic][:, s, itl * P:(itl + 1) * P],
                           wv_sb[:, s, :], s == 0, s == S_D - 1)
                    nc.vector.tensor_copy(
                        v_sb[:, tt, :, 0:HD],
                        ps.rearrange("p (h d) -> p h d", d=HD))

                def normalize(pend):
                    ps_y, h, m, po, ic, ytile = pend
                    rec = rec_pool.tile([1, IC], F32, tag="rec", name="rec")
                    nc.vector.reciprocal(rec[:], ps_y[HD:HD + 1, :])
                    recb = recb_pool.tile([HD, IC], F32, tag="recb",
                                          name="recb")
                    nc.gpsimd.partition_broadcast(recb[:], rec[:])
                    nc.vector.tensor_mul(ytile[po:po + HD, m, :],
                                         ps_y[0:HD, :], recb[:])

                def out_proj(ic, ytile):
                    for itl in range(JPC):
                        for ec in range(NEC):
                            ps_o = psum_g.tile([P, 512], F32, tag="psg",
                                               name="psg")
                            for m in range(NPAIR):
                                mm(ps_o, ytile[:, m, itl * P:(itl + 1) * P],
                                   wo_sb[:, m, ec * 512:(ec + 1) * 512],
                                   m == 0, m == NPAIR - 1)
                            o_t = out_pool.tile([P, 512], F32, tag="ot",
                                                name="ot")
                            nc.vector.tensor_copy(o_t[:], ps_o)
                            nc.sync.dma_start(
                                out_r[:, ic * JPC + itl,
                                      ec * 512:(ec + 1) * 512], o_t[:])

                # phase 1 for chunk 0 (nothing to overlap with yet)
                qTc[0] = qT_pool.tile([P, NPAIR, IC], MMD, tag="qT", name="qT")
                for m in range(NPAIR):
                    group_q(0, m)
                for m in range(NPAIR):
                    group_k(0, m)
                for itl in range(JPC):
                    group_v(0, itl)

                pending = None
                for ic in range(NIC):
                    # prefetch next x chunk; its k/v projection groups
                    # interleave into this attention block below
                    kv_todo = []
                    if ic + 1 < NIC:
                        nxc = x_pool.tile([P, S_D, IC], MMD, tag="xc",
                                          name="xc")
                        xcs[ic + 1] = nxc
                        nc.sync.dma_start(
                            nxc[:], xT_r[:, :, (ic + 1) * IC:(ic + 2) * IC])
                        kv_todo = ([lambda m=m: group_k(ic + 1, m)
                                    for m in range(NPAIR)]
                                   + [lambda i=i: group_v(ic + 1, i)
                                      for i in range(JPC)])
                    h0 = h0_0 if ic == 0 else h0_n
                    ytile = yT_pool.tile([P, NPAIR, IC], MMD, tag="yT",
                                         name="yT")

                    if skip_attention:  # timing probe: phases 1+3 only
                        nc.vector.tensor_copy(
                            ytile[:], kT[:, :, ic * IC:(ic + 1) * IC])
                        for fn in kv_todo:
                            fn()
                        if ic + 1 < NIC:
                            qTc[ic + 1] = qT_pool.tile([P, NPAIR, IC], MMD,
                                                       tag="qT", name="qT")
                            for m in range(NPAIR):
                                group_q(ic + 1, m)
                        out_proj(ic, ytile)
                        continue
                    for h in range(HLOC):
                        m, po = h // 2, (h % 2) * HD
                        njb = JPC * (ic + 1)
                        ps_y = (psum_g if share_psy else psum_y).tile(
                            [HD + 1, IC], F32,
                            tag="psg" if share_psy else "psy", name="psy")
                        # units pack 2 j-tiles per PSUM double-bank so one
                        # ACTIVATE covers both.
                        # Each entry: (masked, [(psum_off, i_off, width, jb)..])
                        units = []
                        for p2 in range(ic * JPC // 2):
                            units.append((False, [(0, 0, IC, 2 * p2),
                                                  (IC, 0, IC, 2 * p2 + 1)]))
                        d = ic * JPC
                        units.append((True, [(0, 0, 512, d),
                                             (512, P, 384, d + 1)]))
                        units.append((True, [(0, 2 * P, 256, d + 2),
                                             (256, 3 * P, 128, d + 3)]))
                        work = []  # (pt2, parts)
                        for ui, (msk, parts) in enumerate(units):
                            ps2 = psum_s.tile([P, 2 * IC], F32, tag="pss",
                                              name="pss")
                            for (off, i0, wdt, jb) in parts:
                                r_off = max(0, (jb - JPC * ic) * P)
                                mm(ps2[:, off:off + wdt],
                                   kT[po:po + HD, m, jb * P:(jb + 1) * P],
                                   qTc[ic][po:po + HD, m, r_off:IC],
                                   True, True)
                            ext = parts[-1][0] + parts[-1][2]
                            pt2 = pt_pool.tile([P, 2 * IC], MMD, tag="pt",
                                               name="pt")
                            nc.scalar.activation(
                                pt2[:, 0:ext], ps2[:, 0:ext],
                                mybir.ActivationFunctionType.Exp, scale=scale)
                            if msk:
                                # only the first 128 cols of each diagonal
                                # part are causally partial
                                for (off, i0, wdt, jb) in parts:
                                    nc.vector.tensor_mul(
                                        pt2[:, off:off + P],
                                        pt2[:, off:off + P], stair[:])
                            work.append((pt2, parts))
                            if ui == 0 and pending is not None:
                                # out-proj of the previous chunk, deferred so
                                # its matmuls don't wait on the DVE normalize
                                out_proj(pending[0], pending[1])
                                pending = None
                            if len(work) > pv_lag:
                                ppt, pparts = work[-1 - pv_lag]
                                for (off, i0, wdt, jb) in pparts:
                                    mm(ps_y[:, IC - wdt:IC],
                                       v_sb[:, jb, h, :], ppt[:, off:off + wdt],
                                       jb == 0, False)
                        for (ppt, pparts) in work[-min(pv_lag, len(work)):]:
                            for (off, i0, wdt, jb) in pparts:
                                mm(ps_y[:, IC - wdt:IC], v_sb[:, jb, h, :],
                                   ppt[:, off:off + wdt], jb == 0,
                                   jb == njb - 1)
                        normalize((ps_y, h, m, po, ic, ytile))
                        if h == HLOC - 1:
                            pending = (ic, ytile)
                        # interleave next chunk's k/v projection groups
                        if h >= h0 and kv_todo:
                            n_left = HLOC - h
                            n_emit = max(1, -(-len(kv_todo) // n_left))
                            for _ in range(min(n_emit, len(kv_todo))):
                                kv_todo.pop(0)()
                    for fn in kv_todo:
                        fn()
                    if ic + 1 < NIC:  # q projection for next chunk
                        qTc[ic + 1] = qT_pool.tile([P, NPAIR, IC], MMD,
                                                   tag="qT", name="qT")
                        for m in range(NPAIR):
                            group_q(ic + 1, m)
                if not skip_attention:
                    out_proj(pending[0], pending[1])
                    pending = None

    nc.compile()
    return nc


def shard_inputs(x, w_qkv, w_out):
    """Full inputs -> list of 8 per-core input maps (bf16)."""
    x = np.asarray(x, dtype=np.float32)
    w_qkv = np.asarray(w_qkv, dtype=np.float32)
    w_out = np.asarray(w_out, dtype=np.float32)
    in_maps = []
    for c in range(N_CORES):
        b, g = c // 2, c % 2
        hsl = slice(g * W, (g + 1) * W)
        in_maps.append({
            "xT": np.ascontiguousarray(x[b].T).astype(NP_BF16),
            "wq": np.ascontiguousarray(
                w_qkv[:, 0 * D:1 * D][:, hsl]).astype(NP_BF16),
            "wk": np.ascontiguousarray(
                w_qkv[:, 1 * D:2 * D][:, hsl]).astype(NP_BF16),
            "wv": np.ascontiguousarray(
                w_qkv[:, 2 * D:3 * D][:, hsl]).astype(NP_BF16),
            "wo": np.ascontiguousarray(w_out[hsl, :]).astype(NP_BF16),
        })
    return in_maps


_NC_CACHE = {}


def _run_spmd(nc, in_maps):
    """One execution on cores 0-7. First call goes through
    run_bass_kernel_spmd (which jits + compiles the NEFF via the axon/PJRT
    path); the compiled executable is cached so repeat kernel() calls skip
    the multi-minute recompile."""
    import os
    os.environ.setdefault("BASS_NEVER_TRACE", "1")  # no NTFF hook here
    fn_pack = _NC_CACHE.get("fn")
    if fn_pack is None:
        import jax
        from jax.sharding import Mesh, PartitionSpec, NamedSharding
        try:
            from jax.experimental.shard_map import shard_map
        except ImportError:
            from jax import shard_map
        from concourse import bass2jax

        bass2jax.install_neuronx_cc_hook()
        pname = nc.partition_id_tensor.name if nc.partition_id_tensor else None
        in_names, out_names, out_avals = [], [], []
        for alloc in nc.m.functions[0].allocations:
            if not isinstance(alloc, mybir.MemoryLocationSet):
                continue
            name = alloc.memorylocations[0].name
            if alloc.kind == "ExternalInput":
                if name != pname:
                    in_names.append(name)
            elif alloc.kind == "ExternalOutput":
                out_names.append(name)
                out_avals.append(jax.core.ShapedArray(
                    tuple(alloc.tensor_shape), mybir.dt.np(alloc.dtype)))
        all_in = list(in_names) + list(out_names) + ([pname] if pname else [])

        def _body(*args):
            operands = list(args)
            if pname is not None:
                operands.append(bass2jax.partition_id_tensor())
            return tuple(bass2jax._bass_exec_p.bind(
                *operands, out_avals=tuple(out_avals),
                in_names=tuple(all_in), out_names=tuple(out_names),
                lowering_input_output_aliases=(),
                sim_require_finite=True, sim_require_nnan=True, nc=nc))

        mesh = Mesh(np.asarray(jax.devices()[:N_CORES]), ("core",))
        n_io = len(in_names) + len(out_names)
        fn = jax.jit(shard_map(_body, mesh=mesh,
                               in_specs=(PartitionSpec("core"),) * n_io,
                               out_specs=(PartitionSpec("core"),) * len(out_names),
                               check_rep=False), keep_unused=True)
        sharding = NamedSharding(mesh, PartitionSpec("core"))
        fn_pack = (fn, in_names, out_names, out_avals, sharding, jax)
        _NC_CACHE["fn"] = fn_pack
    fn, in_names, out_names, out_avals, sharding, jax = fn_pack
    concat_in = [np.concatenate([np.asarray(m[n]) for m in in_maps], axis=0)
                 for n in in_names]
    concat_zeros = [np.zeros((N_CORES * a.shape[0], *a.shape[1:]), a.dtype)
                    for a in out_avals]
    dev_args = [jax.device_put(a, sharding) for a in concat_in + concat_zeros]
    outs = fn(*dev_args)
    jax.block_until_ready(outs)
    return [
        {n: np.asarray(outs[i]).reshape(N_CORES, *out_avals[i].shape)[c]
         for i, n in enumerate(out_names)}
        for c in range(N_CORES)
    ]


def kernel(x, w_qkv, w_out, **run_kwargs):
    if "nc" not in _NC_CACHE:
        _NC_CACHE["nc"] = build_nc()
    nc = _NC_CACHE["nc"]
    in_maps = shard_inputs(x, w_qkv, w_out)
    try:
        results = _run_spmd(nc, in_maps)
    except Exception:
        res = run_bass_kernel_spmd(nc, in_maps, core_ids=list(range(N_CORES)),
                                   **run_kwargs)
        _NC_CACHE["last_results"] = res
        results = res.results
    outs = [r["out"] for r in results]
    full = np.stack([outs[2 * b] + outs[2 * b + 1] for b in range(B)], axis=0)
    return full


if __name__ == "__main__":
    rng = np.random.default_rng(0)
    x = rng.standard_normal((B, T, D), dtype=np.float32)
    w_qkv = (rng.standard_normal((D, 3 * D), dtype=np.float32) / np.sqrt(D))
    w_out = (rng.standard_normal((D, D), dtype=np.float32) / np.sqrt(D))
    y = kernel(x, w_qkv, w_out)
    print("out", y.shape, y.dtype, float(np.abs(y).mean()))
